# revision 1
# baseline (speedup 1.0000x reference)
"""CrossNet (3 cross layers) Trainium2 Bass kernel, 8-core data parallel.

Problem: y_{k+1} = y_k * (y_k . w_k) + b_k + y_k, three layers, B=16384 D=2048.

Algebraic restructure (exact): expanding the recurrence,
    y3 = alpha (.) y0  +  beta1 (x) b1  +  beta2 (x) b2  +  1 (x) b3
where the per-row scalars come from three dots d_j = y0 . w_j:
    t0 = 1 + d1
    t1 = 1 + t0*d2 + c12              c12 = b1.w2
    t2 = 1 + t1*(t0*d3 + c13) + c23   c13 = b1.w3, c23 = b2.w3
    alpha = t0*t1*t2, beta1 = t1*t2, beta2 = t2
so the device work per 128-row tile is:
  PE   : transpose y0 blocks (fp32, exact) -> dots W^T yT (float32r)
         -> rank-3 bias-term matmul (float32r)
  ACT  : PSUM->SBUF copies (round to f32r)
  DVE  : tiny scalar pipeline + one fused pass  out = (y0 * alpha) + R_psum
Each input/output byte moves exactly once: the kernel is HBM-bound.

Sharding: pure data parallel, batch split 8 x 2048 rows; the six [D,1]
parameters are replicated (reduced to W/B matrices + 3 scalars host-side).
"""
import sys
sys.path.insert(0, '/opt/trn_rl_repo')

import numpy as np

B, D = 16384, 2048
N_CORES = 8
ROWS = B // N_CORES
GROUP = 4                 # tiles per group (dot matmul N = 512)
LAG = 2                   # chunks the dot matmul trails the transposes by

_STATE = {}


def _build_program():
    import concourse.bass as bass
    import concourse.tile as tile
    from concourse import bacc, mybir

    F32 = mybir.dt.float32
    F32R = mybir.dt.float32r
    MULT = mybir.AluOpType.mult
    ADD = mybir.AluOpType.add

    NT = ROWS // 128          # 128-row tiles per core
    NG = NT // GROUP          # tile groups
    NC = D // 128             # 128-feature chunks
    NCH = D // 512            # 512-feature chunks
    GN = GROUP * 128          # batch cols per group

    nc = bacc.Bacc("TRN2", target_bir_lowering=False, debug=False,
                   num_devices=N_CORES)
    x = nc.dram_tensor("x", [ROWS, D], F32, kind="ExternalInput")
    wmat = nc.dram_tensor("wmat", [D, 3], F32, kind="ExternalInput")
    bmat = nc.dram_tensor("bmat", [3, D], F32, kind="ExternalInput")
    kvec = nc.dram_tensor("kvec", [128, 3], F32, kind="ExternalInput")
    idm = nc.dram_tensor("idm", [128, 128], F32, kind="ExternalInput")
    y = nc.dram_tensor("y", [ROWS, D], F32, kind="ExternalOutput")

    with tile.TileContext(nc) as tc:
        with (
            tc.tile_pool(name="const", bufs=1) as constp,
            tc.tile_pool(name="xin", bufs=3 * GROUP) as xin,
            tc.tile_pool(name="outp", bufs=GROUP + 1) as outp,
            tc.tile_pool(name="yt", bufs=4) as ytp,
            tc.tile_pool(name="sbs", bufs=4) as sbs,
            tc.tile_pool(name="tp", bufs=2, space=bass.MemorySpace.PSUM) as tpp,
            tc.tile_pool(name="dp", bufs=2, space=bass.MemorySpace.PSUM) as dpp,
            tc.tile_pool(name="sp", bufs=2, space=bass.MemorySpace.PSUM) as spp,
            tc.tile_pool(name="rp", bufs=2, space=bass.MemorySpace.PSUM) as rpp,
        ):
            # ---- constants ----
            id_sb = constp.tile([128, 128], F32)
            nc.sync.dma_start(id_sb[:], idm.ap())
            w_sb = constp.tile([128, 3 * NC], F32)
            nc.sync.dma_start(
                w_sb[:].rearrange("p (c j) -> p c j", j=3),
                wmat.ap().rearrange("(c p) j -> p c j", p=128))
            w_r = constp.tile([128, 3 * NC], F32R)
            nc.vector.tensor_copy(w_r[:], w_sb[:])
            bst = constp.tile([3, D], F32)
            nc.sync.dma_start(bst[:], bmat.ap())
            bst_r = constp.tile([3, D], F32R)
            nc.vector.tensor_copy(bst_r[:], bst[:])
            kv = constp.tile([128, 3], F32)
            nc.sync.dma_start(kv[:], kvec.ap())

            for g in range(NG):
                base = g * GROUP
                # ---- load the group's y0 tiles ----
                xt = []
                for t in range(GROUP):
                    r0 = (base + t) * 128
                    tl = xin.tile([128, D], F32, tag="xin")
                    nc.sync.dma_start(tl[:], x.ap()[r0:r0 + 128, :])
                    xt.append(tl)
                # ---- transposes + dots (dot matmul trails by LAG chunks) ----
                d_ps = dpp.tile([3, GN], F32)
                ytcs = {}
                for c in range(NC + LAG):
                    if c < NC:
                        tp = tpp.tile([128, GN], F32)
                        for t in range(GROUP):
                            nc.tensor.transpose(
                                tp[:, t * 128:(t + 1) * 128],
                                xt[t][:, c * 128:(c + 1) * 128],
                                id_sb[:])
                        ytc = ytp.tile([128, GN], F32R)
                        nc.scalar.copy(ytc[:], tp[:])
                        ytcs[c] = ytc
                    cd = c - LAG
                    if cd >= 0:
                        nc.tensor.matmul(
                            d_ps[:], w_r[:, 3 * cd:3 * cd + 3],
                            ytcs.pop(cd)[:],
                            start=(cd == 0), stop=(cd == NC - 1))
                # ---- d -> transposed [128, 3*GROUP] ----
                d_sb = sbs.tile([3, GN], F32, tag="dsb")
                nc.scalar.copy(d_sb[:], d_ps[:])
                dT_ps = spp.tile([128, 3 * GROUP], F32, tag="sp")
                for t in range(GROUP):
                    nc.tensor.transpose(
                        dT_ps[:, 3 * t:3 * t + 3],
                        d_sb[:, t * 128:(t + 1) * 128],
                        id_sb[:3, :3])
                dT = sbs.tile([128, 3 * GROUP], F32, tag="dT")
                nc.scalar.copy(dT[:], dT_ps[:])
                # ---- scalar pipeline on [128, GROUP] ----
                d1, d2, d3 = dT[:, 0::3], dT[:, 1::3], dT[:, 2::3]
                pipe = sbs.tile([128, 6 * GROUP], F32, tag="pipe")
                t0 = pipe[:, 0 * GROUP:1 * GROUP]
                t1 = pipe[:, 1 * GROUP:2 * GROUP]
                t2 = pipe[:, 2 * GROUP:3 * GROUP]
                b1c = pipe[:, 3 * GROUP:4 * GROUP]
                al = pipe[:, 4 * GROUP:5 * GROUP]
                tmp = pipe[:, 5 * GROUP:6 * GROUP]
                nc.vector.tensor_scalar_add(t0, d1, 1.0)
                nc.vector.tensor_tensor(tmp, t0, d2, op=MULT)
                nc.vector.tensor_scalar_add(t1, tmp, kv[:, 0:1])
                nc.vector.tensor_tensor(tmp, t0, d3, op=MULT)
                nc.vector.tensor_scalar_add(tmp, tmp, kv[:, 1:2])
                nc.vector.tensor_tensor(tmp, t1, tmp, op=MULT)
                nc.vector.tensor_scalar_add(t2, tmp, kv[:, 2:3])
                nc.vector.tensor_tensor(b1c, t1, t2, op=MULT)
                nc.vector.tensor_tensor(al, t0, b1c, op=MULT)
                # ---- beta matrices for all tiles (hoisted) ----
                bx = sbs.tile([128, 3 * GROUP], F32, tag="bx")
                for t in range(GROUP):
                    nc.vector.tensor_copy(bx[:, 3 * t:3 * t + 1],
                                          b1c[:, t:t + 1])
                    nc.vector.tensor_copy(bx[:, 3 * t + 1:3 * t + 2],
                                          t2[:, t:t + 1])
                nc.vector.memset(bx[:, 2::3], 1.0)
                bms = []
                for t in range(GROUP):
                    bm_ps = spp.tile([3, 128], F32, tag="sp")
                    nc.tensor.transpose(bm_ps[:], bx[:, 3 * t:3 * t + 3],
                                        id_sb[:])
                    bm = sbs.tile([3, 128], F32R, tag="bm")
                    nc.scalar.copy(bm[:], bm_ps[:])
                    bms.append(bm)
                # ---- per tile: rank-3 bias matmul + fused combine ----
                for t in range(GROUP):
                    ot = outp.tile([128, D], F32, tag="outp")
                    for ch in range(NCH):
                        rp_t = rpp.tile([128, 512], F32)
                        nc.tensor.matmul(
                            rp_t[:], bms[t][:],
                            bst_r[:, 512 * ch:512 * (ch + 1)],
                            start=True, stop=True)
                        nc.vector.scalar_tensor_tensor(
                            ot[:, 512 * ch:512 * (ch + 1)],
                            xt[t][:, 512 * ch:512 * (ch + 1)],
                            al[:, t:t + 1],
                            rp_t[:],
                            op0=MULT, op1=ADD)
                    r0 = (base + t) * 128
                    nc.gpsimd.dma_start(y.ap()[r0:r0 + 128, :], ot[:])
    nc.compile()
    return nc


def _make_runner(nc):
    """Jitted 8-core SPMD executor (mirrors bass2jax.run_bass_via_pjrt)."""
    import jax
    from jax.sharding import Mesh, PartitionSpec, NamedSharding
    from jax.experimental.shard_map import shard_map
    from concourse import bass2jax, mybir

    bass2jax.install_neuronx_cc_hook()
    partition_name = (nc.partition_id_tensor.name
                      if nc.partition_id_tensor else None)
    in_names, out_names, out_avals = [], [], []
    for alloc in nc.m.functions[0].allocations:
        if not isinstance(alloc, mybir.MemoryLocationSet):
            continue
        name = alloc.memorylocations[0].name
        if alloc.kind == "ExternalInput":
            if name != partition_name:
                in_names.append(name)
        elif alloc.kind == "ExternalOutput":
            out_names.append(name)
            out_avals.append(jax.core.ShapedArray(
                tuple(alloc.tensor_shape), mybir.dt.np(alloc.dtype)))
    n_params = len(in_names)
    all_in_names = list(in_names) + out_names
    if partition_name is not None:
        all_in_names.append(partition_name)

    def _body(*args):
        operands = list(args)
        if partition_name is not None:
            operands.append(bass2jax.partition_id_tensor())
        return tuple(bass2jax._bass_exec_p.bind(
            *operands,
            out_avals=tuple(out_avals),
            in_names=tuple(all_in_names),
            out_names=tuple(out_names),
            lowering_input_output_aliases=(),
            sim_require_finite=True,
            sim_require_nnan=True,
            nc=nc,
        ))

    devices = jax.devices()[:N_CORES]
    mesh = Mesh(np.asarray(devices), ("core",))
    spec = NamedSharding(mesh, PartitionSpec("core"))
    n_args = n_params + len(out_names)
    fn = jax.jit(shard_map(
        _body, mesh=mesh,
        in_specs=(PartitionSpec("core"),) * n_args,
        out_specs=(PartitionSpec("core"),) * len(out_names)))
    return fn, spec, in_names, out_names, out_avals


def _get_state():
    if not _STATE:
        nc = _build_program()
        fn, spec, in_names, out_names, out_avals = _make_runner(nc)
        _STATE.update(fn=fn, spec=spec, in_names=in_names,
                      out_names=out_names, out_avals=out_avals)
    return _STATE


def _host_prep(cross_input, weight1, bias1, weight2, bias2, weight3, bias3):
    """Full inputs -> dict of per-core-stacked input arrays."""
    w = [np.asarray(weight1, np.float32), np.asarray(weight2, np.float32),
         np.asarray(weight3, np.float32)]
    b = [np.asarray(bias1, np.float32), np.asarray(bias2, np.float32),
         np.asarray(bias3, np.float32)]
    wmat = np.concatenate([wi.reshape(D, 1) for wi in w], axis=1)
    bmat = np.stack([bi.reshape(D) for bi in b], axis=0)
    c12 = float(b[0].reshape(D).astype(np.float64)
                @ w[1].reshape(D).astype(np.float64))
    c13 = float(b[0].reshape(D).astype(np.float64)
                @ w[2].reshape(D).astype(np.float64))
    c23 = float(b[1].reshape(D).astype(np.float64)
                @ w[2].reshape(D).astype(np.float64))
    kvec = np.tile(np.array([1.0 + c12, c13, 1.0 + c23], np.float32),
                   (128, 1))
    idm = np.eye(128, dtype=np.float32)
    xs = np.ascontiguousarray(np.asarray(cross_input, np.float32))
    reps = {"wmat": wmat, "bmat": bmat, "kvec": kvec, "idm": idm}
    # stack per-core inputs along axis 0 (shard_map splits on "core")
    full = {"x": xs}
    for k, v in reps.items():
        full[k] = np.concatenate([v] * N_CORES, axis=0)
    return full


def kernel(cross_input, weight1, bias1, weight2, bias2, weight3, bias3):
    import jax
    st = _get_state()
    full = _host_prep(cross_input, weight1, bias1, weight2, bias2,
                      weight3, bias3)
    args = [jax.device_put(full[nm], st["spec"]) for nm in st["in_names"]]
    zeros = [jax.device_put(
        np.zeros((N_CORES * av.shape[0], *av.shape[1:]), av.dtype),
        st["spec"]) for av in st["out_avals"]]
    outs = st["fn"](*args, *zeros)
    out = np.asarray(outs[st["out_names"].index("y")])
    return out.reshape(B, D)
